# revision 33
# baseline (speedup 1.0000x reference)
"""Trainium2 Bass kernel for nn_MistralMoLoraLayer (MoE-routed LoRA FFN).

Strategy: data-parallel over tokens (8 cores x 256 tokens), base FFN weights
replicated, all-expert LoRA replicated. The per-(batch,slot) softmax over the
sequence axis needs denominators summed over the whole batch; each core
receives the OTHER 768 tokens of its batch (xr) and computes the exp-sums
locally -- no collective, so cores never sync (start-skew stays out of every
core's span).

All heavy GEMMs run in bf16 (router logits stay f32 so top-2 selection is
tie-exact vs the f32 reference; rest-of-batch logits feed only the exp-sum
denominator and run f32r). DMA instruction count is minimized via host-side
packed layouts (sequencer + HWDGE cost ~0.6us per DMA): one DMA for x, one
per h-tile for wu|wg, one per h-tile-PAIR for uB|gB|dA, one per d-tile row
of w_down. The router stream (xr/xTr/gw/A) issues on the Activation queue so
it does not delay the weight stream on the SP queue.

Per-core math (all tiles [h/er/d partitions, tokens free]):
  router: logits = x @ gate_w.T; top-2 (value,index) per token; exp;
          denom[slot] = sum over own 256 + rest 768 tokens of exp;
          weights w_j = exp_j / denom[slot]
  A-proj: UA/GA [E*R=128, t] = stacked up_A/gate_A @ x.T   (one K=128 chain)
  slot-mask trick: Ut_j = UA * M_j where M_j[e*R+r, t] = (sel_j(t)==e);
          lo_up_j[h,t] = (stacked up_B) @ Ut_j  == up_B[sel_j(t)] @ u_{sel_j(t)}
  h_j = silu(U + lo_up_j) * (G + lo_gate_j); ch_j = c_j * h_j
  mixed = ch_0 + ch_1
  v_j[er,t] = (stacked down_A) @ ch_j  (accumulated over h), masked by M_j
  outT[d,t] = w_down-chain @ mixed + (stacked down_B) @ v_0 + ... @ v_1
"""

import numpy as np

# problem constants (hardcoded; kernel.py must be self-contained)
B, S, D, H, E, R, TOPK = 2, 1024, 2048, 5632, 8, 16, 2
ALPHA = 2.0
T = B * S
NCORES = 8
TC = T // NCORES           # 256 tokens per core
KT = D // 128              # 16 k-tiles over D
HT = H // 128              # 44 h-tiles
DT = D // 128              # 16 d-tiles
ER = E * R                 # 128
SR = S - TC                # 768 rest-of-batch tokens for local denom sums
import os as _os
NHOIST = int(_os.environ.get("KNHOIST", "2"))  # h-loop software-pipeline lag
NDEV = int(_os.environ.get("KNDEV", "1"))      # devices declared in the NEFF

_cache = {}


def _build():
    import concourse.bacc as bacc
    import concourse.bass as bass
    import concourse.mybir as mybir
    import concourse.tile as tile
    from concourse.masks import make_identity

    f32 = mybir.dt.float32
    bf16 = mybir.dt.bfloat16
    AL = mybir.AluOpType
    AF = mybir.ActivationFunctionType

    def fr(ap):
        # f32-stored operand viewed as f32r for fast full-precision-ish matmul
        return ap.bitcast(mybir.dt.float32r)

    # no collectives and no partition-id use remain: build a single-device
    # program (8 independent copies run via shard_map; avoids any comm-group
    # setup at NEFF load)
    nc = bacc.Bacc("TRN2", target_bir_lowering=False, debug=False,
                   num_devices=NDEV)

    # ---- DRAM I/O (host-prepped packed layouts) ----
    d_xT = nc.dram_tensor("xT", [128, KT * TC], bf16, kind="ExternalInput").ap()
    d_xTr = nc.dram_tensor("xTr", [128, KT * TC], f32,
                           kind="ExternalInput").ap()
    d_xr = nc.dram_tensor("xr", [128, KT * SR], bf16,
                          kind="ExternalInput").ap()
    d_gw = nc.dram_tensor("gw", [128, KT * E], f32, kind="ExternalInput").ap()
    d_gwb = nc.dram_tensor("gwb", [128, KT * E], bf16,
                           kind="ExternalInput").ap()
    d_wug = nc.dram_tensor("wug", [HT, 128, 2 * KT * 128], bf16,
                           kind="ExternalInput").ap()
    d_ubgd = nc.dram_tensor("ubgd", [HT // 2, 128, 768], bf16,
                            kind="ExternalInput").ap()
    d_wd = nc.dram_tensor("wd", [DT, 128, HT * 128], bf16,
                          kind="ExternalInput").ap()
    d_A = nc.dram_tensor("Ah", [128, KT * 2 * ER], bf16,
                         kind="ExternalInput").ap()
    d_dB = nc.dram_tensor("dB", [128, D], bf16, kind="ExternalInput").ap()
    d_eid = nc.dram_tensor("eid", [128, 1], f32, kind="ExternalInput").ap()
    d_i8m = nc.dram_tensor("i8m", [128, E], f32, kind="ExternalInput").ap()
    d_sel2 = nc.dram_tensor("sel2", [2, 256], f32, kind="ExternalInput").ap()
    d_out = nc.dram_tensor("outT", [D, TC], f32, kind="ExternalOutput").ap()

    with tile.TileContext(nc) as tc:
        import contextlib
        ctx = contextlib.ExitStack()
        with ctx:
            cpool = ctx.enter_context(tc.tile_pool(name="const", bufs=1))
            wpool = ctx.enter_context(tc.tile_pool(name="wstream", bufs=2))
            spool = ctx.enter_context(tc.tile_pool(name="work", bufs=2))
            pspool = ctx.enter_context(
                tc.tile_pool(name="ps", bufs=1, space="PSUM"))

            # ---- DMA: weight stream on SP queue, router stream on Act ----
            xT_sb = cpool.tile([128, KT * TC], bf16, name="xT_sb")
            nc.sync.dma_start(out=xT_sb[:], in_=d_xT[:])

            XRC = 4                       # xr chunks of 4 k-tiles each
            gw_sb = cpool.tile([128, KT * E], f32, name="gw_sb")
            nc.scalar.dma_start(out=gw_sb[:], in_=d_gw[:])
            gwr_sb = cpool.tile([128, KT * E], bf16, name="gwr_sb")
            nc.scalar.dma_start(out=gwr_sb[:], in_=d_gwb[:])
            xr_t = [wpool.tile([128, 4 * SR], bf16, tag="xr", bufs=2,
                               name=f"xr_t{c}") for c in range(XRC)]
            for c in range(XRC):
                nc.scalar.dma_start(
                    out=xr_t[c][:],
                    in_=d_xr[:, c * 4 * SR:(c + 1) * 4 * SR])
            xTr_sb = cpool.tile([128, KT * TC], f32, name="xTr_sb")
            nc.scalar.dma_start(out=xTr_sb[:], in_=d_xTr[:])
            eid_sb = cpool.tile([128, 1], f32, name="eid_sb")
            nc.scalar.dma_start(out=eid_sb[:], in_=d_eid[:])
            i8m_sb = cpool.tile([128, E], f32, name="i8m_sb")
            nc.scalar.dma_start(out=i8m_sb[:], in_=d_i8m[:])
            sel2_sb = cpool.tile([2, 256], f32, name="sel2_sb")
            nc.scalar.dma_start(out=sel2_sb[:], in_=d_sel2[:])
            dB_sb = cpool.tile([128, D], bf16, name="dB_sb")
            nc.scalar.dma_start(out=dB_sb[:], in_=d_dB[:])

            ident = cpool.tile([128, 128], f32, name="ident")
            make_identity(nc, ident)
            ones_col = cpool.tile([128, 1], f32, name="ones_col")
            nc.vector.memset(ones_col, 1.0)

            mixed = cpool.tile([128, HT * TC], bf16, name="mixed")
            ev_rows = cpool.tile([2, TC], f32, name="ev_rows")
            s_rows = cpool.tile([2, TC], f32, name="s_rows")
            crows = cpool.tile([2, TC], f32, name="crows")
            cb = cpool.tile([128, 2 * TC], bf16, name="cb")
            Mj = cpool.tile([128, 2 * TC], bf16, name="Mj")
            UA = cpool.tile([128, TC], bf16, name="UA")
            GA = cpool.tile([128, TC], bf16, name="GA")
            Ut = cpool.tile([128, 2 * TC], bf16, name="Ut")
            Gt = cpool.tile([128, 2 * TC], bf16, name="Gt")
            vt = cpool.tile([128, 2 * TC], bf16, name="vt")
            Lr = cpool.tile([8, SR], f32, name="Lr")
            ev_acc = cpool.tile([128, 2], f32, name="ev_acc")

            def load_wug(i):
                t = wpool.tile([128, 2 * KT * 128], bf16, tag="wug", bufs=3,
                               name="wug_t")
                nc.sync.dma_start(out=t[:], in_=d_wug[i])
                return t

            def load_ubgd(p):
                t = wpool.tile([128, 768], bf16, tag="ubgd", bufs=3,
                               name="ubgd_t")
                nc.sync.dma_start(out=t[:], in_=d_ubgd[p])
                return t

            def base_gemm(i, wug_t):
                # psUG[:, 0:TC] = up, [TC:2TC] = gate for h-tile i
                psUG = pspool.tile([128, 2 * TC], f32, tag="psUG", bufs=2,
                                   name="psUG")
                for k in range(KT):
                    nc.tensor.matmul(psUG[:, 0:TC],
                                     wug_t[:, k * 128:(k + 1) * 128],
                                     xT_sb[:, k * TC:(k + 1) * TC],
                                     start=(k == 0), stop=(k == KT - 1))
                for k in range(KT):
                    nc.tensor.matmul(
                        psUG[:, TC:2 * TC],
                        wug_t[:, (KT + k) * 128:(KT + k + 1) * 128],
                        xT_sb[:, k * TC:(k + 1) * TC],
                        start=(k == 0), stop=(k == KT - 1))
                U_sb = spool.tile([128, TC], bf16, tag="U_sb",
                                  bufs=NHOIST + 4, name="U_sb")
                nc.scalar.copy(U_sb[:], psUG[:, 0:TC])
                G_sb = spool.tile([128, TC], bf16, tag="G_sb",
                                  bufs=NHOIST + 4, name="G_sb")
                nc.scalar.copy(G_sb[:], psUG[:, TC:2 * TC])
                return U_sb, G_sb

            # ---- hoisted base GEMMs: keep PE busy from the first us while
            #      the router inputs stream in on the Act queue ----
            ug_done = {}                # h-tile -> (U_sb, G_sb)
            ubgd_pre = {}
            wug_pre = [load_wug(i) for i in range(min(2, HT))]
            A_sb = cpool.tile([128, KT * 2 * ER], bf16, name="A_sb")
            nc.sync.dma_start(out=A_sb[:], in_=d_A[:])
            for p in range(2):
                ubgd_pre[p] = load_ubgd(p)
            for i in range(min(2, HT)):
                ug_done[i] = base_gemm(i, wug_pre[i])

            # ---- stacked A-projections (independent of the router; keeps
            #      PE fed while xr/xTr stream in) ----
            psUA = pspool.tile([128, TC], f32, tag="psUG", bufs=2, name="psUA")
            for k in range(KT):
                nc.tensor.matmul(psUA[:],
                                 A_sb[:, k * 2 * ER: k * 2 * ER + ER],
                                 xT_sb[:, k * TC:(k + 1) * TC],
                                 start=(k == 0), stop=(k == KT - 1))
            nc.vector.tensor_copy(UA[:], psUA[:])
            psGA = pspool.tile([128, TC], f32, tag="psUG", bufs=2, name="psGA")
            for k in range(KT):
                nc.tensor.matmul(psGA[:],
                                 A_sb[:, k * 2 * ER + ER:(k + 1) * 2 * ER],
                                 xT_sb[:, k * TC:(k + 1) * TC],
                                 start=(k == 0), stop=(k == KT - 1))
            nc.vector.tensor_copy(GA[:], psGA[:])

            # ---- phase 1a: rest-of-batch logits (bf16, denom-only), E x SR --
            RH = SR // 2
            psr_a = pspool.tile([8, RH], f32, tag="psUG", bufs=2, name="psr_a")
            psr_b = pspool.tile([8, RH], f32, tag="psUG", bufs=2, name="psr_b")
            for k in range(KT):
                xc = xr_t[k // 4]
                sl = (k % 4) * SR
                nc.tensor.matmul(psr_a[:], gwr_sb[:, k * E:(k + 1) * E],
                                 xc[:, sl:sl + RH],
                                 start=(k == 0), stop=(k == KT - 1))
                nc.tensor.matmul(psr_b[:], gwr_sb[:, k * E:(k + 1) * E],
                                 xc[:, sl + RH:sl + SR],
                                 start=(k == 0), stop=(k == KT - 1))
            nc.vector.tensor_copy(Lr[:, 0:RH], psr_a[:])
            nc.vector.tensor_copy(Lr[:, RH:SR], psr_b[:])

            # ---- phase 1b: own-token dance (full f32 logits) ----
            for tt in range(2):
                psL = pspool.tile([128, TC], f32, tag="ps_small", name="psL")
                for k in range(KT):
                    nc.tensor.matmul(
                        psL[:, 0:E],
                        xTr_sb[:, k * TC + tt * 128: k * TC + tt * 128 + 128],
                        gw_sb[:, k * E:(k + 1) * E],
                        start=(k == 0), stop=(k == KT - 1))
                L = spool.tile([128, E], f32, tag="L")
                nc.vector.tensor_copy(L[:], psL[:, 0:E])
                mx1 = spool.tile([128, 1], f32, tag="mx1")
                nc.vector.tensor_reduce(mx1[:], L[:], mybir.AxisListType.X,
                                        AL.max)
                msk = spool.tile([128, E], f32, tag="msk")
                nc.vector.tensor_scalar(msk[:], L[:], mx1[:], None,
                                        AL.is_equal)
                mi = spool.tile([128, E], f32, tag="mi")
                nc.vector.tensor_tensor(mi[:], msk[:], i8m_sb[:], AL.mult)
                svals = spool.tile([128, 2], f32, tag="svals")
                nc.vector.tensor_reduce(svals[:, 0:1], mi[:],
                                        mybir.AxisListType.X, AL.max)
                evals = spool.tile([128, 2], f32, tag="evals")
                nc.scalar.activation(evals[:, 0:1], mx1[:], AF.Exp)
                # mask out slot-0 winner, find second max
                big = spool.tile([128, E], f32, tag="big")
                nc.vector.tensor_scalar(big[:], msk[:], 1e30, None, AL.mult)
                L2 = spool.tile([128, E], f32, tag="L2")
                nc.vector.tensor_tensor(L2[:], L[:], big[:], AL.subtract)
                mx2 = spool.tile([128, 1], f32, tag="mx2")
                nc.vector.tensor_reduce(mx2[:], L2[:], mybir.AxisListType.X,
                                        AL.max)
                msk2 = spool.tile([128, E], f32, tag="msk2")
                nc.vector.tensor_scalar(msk2[:], L2[:], mx2[:], None,
                                        AL.is_equal)
                mi2 = spool.tile([128, E], f32, tag="mi2")
                nc.vector.tensor_tensor(mi2[:], msk2[:], i8m_sb[:], AL.mult)
                nc.vector.tensor_reduce(svals[:, 1:2], mi2[:],
                                        mybir.AxisListType.X, AL.max)
                nc.scalar.activation(evals[:, 1:2], mx2[:], AF.Exp)
                # accumulate exp sums for the denominator
                if tt == 0:
                    nc.vector.tensor_copy(ev_acc[:], evals[:])
                else:
                    nc.vector.tensor_tensor(ev_acc[:], ev_acc[:], evals[:],
                                            AL.add)
                # transpose evals/svals -> rows
                psT = pspool.tile([2, 128], f32, tag="ps_small", name="psT")
                nc.tensor.transpose(psT[:], evals[:], ident[:])
                nc.vector.tensor_copy(ev_rows[:, tt * 128:(tt + 1) * 128],
                                      psT[:])
                psT2 = pspool.tile([2, 128], f32, tag="ps_small", name="psT2")
                nc.tensor.transpose(psT2[:], svals[:], ident[:])
                nc.vector.tensor_copy(s_rows[:, tt * 128:(tt + 1) * 128],
                                      psT2[:])

            # ---- expert masks + masked A-projections: these depend only on
            #      the own-token dance (selection), NOT on the denominators,
            #      so the h-loop's psLO matmuls unblock before the rest dance
            for j in range(2):
                psM = pspool.tile([128, TC], f32, tag="ps_small", name="psM")
                nc.tensor.matmul(psM[:], sel2_sb[:, j * 128:(j + 1) * 128],
                                 s_rows[:], start=True, stop=True)
                nc.vector.tensor_scalar(Mj[:, j * TC:(j + 1) * TC], psM[:],
                                        eid_sb[:], None, AL.is_equal)
                nc.vector.tensor_tensor(Ut[:, j * TC:(j + 1) * TC], UA[:],
                                        Mj[:, j * TC:(j + 1) * TC], AL.mult)
                nc.vector.tensor_tensor(Gt[:, j * TC:(j + 1) * TC], GA[:],
                                        Mj[:, j * TC:(j + 1) * TC], AL.mult)

            # ---- phase 1c: rest-token dance (denominator only); all six
            #      transposes land in ONE psum tile to avoid a PE<->DVE
            #      ping-pong on the single ps_small buffer ----
            NRT = SR // 128
            psLt = pspool.tile([128, 8 * NRT], f32, tag="ps_small",
                               name="psLt")
            for rt in range(NRT):
                nc.tensor.transpose(psLt[:, 8 * rt:8 * rt + 8],
                                    Lr[:, rt * 128:(rt + 1) * 128],
                                    ident[0:8, 0:8])
            L6 = spool.tile([128, 8 * NRT], f32, tag="L6", bufs=1, name="L6")
            nc.vector.tensor_copy(L6[:], psLt[:])
            for rt in range(NRT):
                L = L6[:, 8 * rt:8 * rt + 8]
                mx1 = spool.tile([128, 1], f32, tag="mx1")
                nc.vector.tensor_reduce(mx1[:], L, mybir.AxisListType.X,
                                        AL.max)
                msk = spool.tile([128, E], f32, tag="msk")
                nc.vector.tensor_scalar(msk[:], L, mx1[:], None,
                                        AL.is_equal)
                evals = spool.tile([128, 2], f32, tag="evals")
                nc.scalar.activation(evals[:, 0:1], mx1[:], AF.Exp)
                big = spool.tile([128, E], f32, tag="big")
                nc.vector.tensor_scalar(big[:], msk[:], 1e30, None, AL.mult)
                L2 = spool.tile([128, E], f32, tag="L2")
                nc.vector.tensor_tensor(L2[:], L, big[:], AL.subtract)
                mx2 = spool.tile([128, 1], f32, tag="mx2")
                nc.vector.tensor_reduce(mx2[:], L2[:], mybir.AxisListType.X,
                                        AL.max)
                nc.scalar.activation(evals[:, 1:2], mx2[:], AF.Exp)
                nc.vector.tensor_tensor(ev_acc[:], ev_acc[:], evals[:], AL.add)

            # ---- phase 1d: denominators, reciprocal, routing weights ----
            psd = pspool.tile([2, 1], f32, tag="ps_small", name="psd")
            nc.tensor.matmul(psd[:], ev_acc[:], ones_col[:],
                             start=True, stop=True)
            rcp = cpool.tile([2, 1], f32, name="rcp")
            nc.vector.reciprocal(rcp[:], psd[:])
            nc.vector.tensor_scalar(crows[:], ev_rows[:], rcp[:], None,
                                    AL.mult)
            # broadcast weight rows along partitions via K=2 matmul with a
            # row-selector constant (sel2[:, j*128:(j+1)*128] has row j = 1)
            for j in range(2):
                psB = pspool.tile([128, TC], f32, tag="ps_small", name="psB")
                nc.tensor.matmul(psB[:], sel2_sb[:, j * 128:(j + 1) * 128],
                                 crows[:], start=True, stop=True)
                nc.vector.tensor_copy(cb[:, j * TC:(j + 1) * TC], psB[:])

            # ---- phases 2+5+6: h-tile loop ----
            psV = pspool.tile([128, 2 * TC], f32, tag="psV", name="psV")
            wd_pre = {}                 # di -> prefetched full-row tile

            def load_wd(di):
                t = wpool.tile([128, HT * 128], bf16, tag="wd", bufs=4,
                               name="wd_t")
                nc.sync.dma_start(out=t[:], in_=d_wd[di])
                return t

            # software pipeline: iteration i runs the base GEMM for h-tile i
            # and the LoRA/elementwise chain for h-tile j, scheduled with a
            # TAPERED lag: large at the start (chains wait on the router, so
            # queue many base GEMMs ahead of them), shrinking to 1 at the
            # end (minimize the un-overlapped chain tail before the down
            # GEMM can start).
            LAG0 = NHOIST
            sched = {}
            for j_ in range(HT):
                lag = max(1, int(round(LAG0 - (LAG0 - 1) * j_ / 40.0)))
                sched.setdefault(min(j_ + lag, HT), []).append(j_)
            pend = {"v": None}

            def chain(j):
                U_sb, G_sb = ug_done.pop(j)
                ub_t = ubgd_pre[j // 2]
                base = (j % 2) * 384
                uB_t = ub_t[:, base:base + 128]
                gB_t = ub_t[:, base + 128:base + 256]
                dA_t = ub_t[:, base + 256:base + 384]

                if pend["v"] is not None:
                    pv_dA, pv_ch = pend["v"]
                    nc.tensor.matmul(psV[:], pv_dA, pv_ch[:],
                                     start=(j == 1), stop=False,
                                     skip_group_check=True)

                psLO = pspool.tile([128, 4 * TC], f32, tag="psLO", bufs=2,
                                   name="psLO")
                # both slots' c*h in ONE tile so the down_A contraction is a
                # single [128,512] matmul per h-tile
                ch_pair = spool.tile([128, 2 * TC], bf16, tag="chp", bufs=3)
                for sj in range(2):
                    nc.tensor.matmul(psLO[:, (2 * sj) * TC:(2 * sj + 1) * TC],
                                     uB_t, Ut[:, sj * TC:(sj + 1) * TC],
                                     start=True, stop=True)
                    nc.tensor.matmul(
                        psLO[:, (2 * sj + 1) * TC:(2 * sj + 2) * TC],
                        gB_t, Gt[:, sj * TC:(sj + 1) * TC],
                        start=True, stop=True)
                    tu = spool.tile([128, TC], bf16, tag="tu")
                    nc.vector.tensor_tensor(
                        tu[:], U_sb[:],
                        psLO[:, (2 * sj) * TC:(2 * sj + 1) * TC], AL.add)
                    su = spool.tile([128, TC], bf16, tag="su")
                    nc.scalar.activation(su[:], tu[:], AF.Silu)
                    tg = spool.tile([128, TC], bf16, tag="tg")
                    nc.vector.tensor_tensor(
                        tg[:], G_sb[:],
                        psLO[:, (2 * sj + 1) * TC:(2 * sj + 2) * TC], AL.add)
                    hh = spool.tile([128, TC], bf16, tag="hh")
                    nc.vector.tensor_tensor(hh[:], su[:], tg[:], AL.mult)
                    nc.vector.tensor_tensor(ch_pair[:, sj * TC:(sj + 1) * TC],
                                            hh[:],
                                            cb[:, sj * TC:(sj + 1) * TC],
                                            AL.mult)
                nc.vector.tensor_tensor(mixed[:, j * TC:(j + 1) * TC],
                                        ch_pair[:, 0:TC],
                                        ch_pair[:, TC:2 * TC], AL.add)
                pend["v"] = (dA_t, ch_pair)
                # prefetch uB/gB/dA two pairs ahead (after pend_v's reader of
                # the recycled buffer has been emitted)
                if j % 2 == 0 and j // 2 + 2 < HT // 2:
                    ubgd_pre[j // 2 + 2] = load_ubgd(j // 2 + 2)

            for i in range(HT + 1):
                if i < HT:
                    if 38 <= i < 41:
                        wd_pre[i - 38] = load_wd(i - 38)  # prefetch 3 wd rows
                    if i >= 2:
                        ug_done[i] = base_gemm(i, load_wug(i))
                for j in sched.get(i, ()):
                    chain(j)

            pv_dA, pv_ch = pend["v"]
            nc.tensor.matmul(psV[:], pv_dA, pv_ch[:],
                             start=False, stop=True, skip_group_check=True)
            # masked v
            for j in range(2):
                nc.vector.tensor_tensor(vt[:, j * TC:(j + 1) * TC],
                                        psV[:, j * TC:(j + 1) * TC],
                                        Mj[:, j * TC:(j + 1) * TC], AL.mult)

            # ---- phase 7: down GEMM + LoRA-down ----
            for di in range(DT):
                wd_t = wd_pre.pop(di) if di in wd_pre else load_wd(di)
                psO = pspool.tile([128, TC], f32, tag="psUG", bufs=2,
                                  name="psO")
                for hk in range(HT):
                    nc.tensor.matmul(psO[:],
                                     wd_t[:, hk * 128:(hk + 1) * 128],
                                     mixed[:, hk * TC:(hk + 1) * TC],
                                     start=(hk == 0), stop=False,
                                     skip_group_check=True)
                nc.tensor.matmul(psO[:], dB_sb[:, di * 128:(di + 1) * 128],
                                 vt[:, 0:TC], start=False, stop=False,
                                 skip_group_check=True)
                nc.tensor.matmul(psO[:], dB_sb[:, di * 128:(di + 1) * 128],
                                 vt[:, TC:2 * TC], start=False, stop=True,
                                 skip_group_check=True)
                o_sb = spool.tile([128, TC], f32, tag="o_sb")
                nc.scalar.copy(o_sb[:], psO[:])
                nc.sync.dma_start(out=d_out[di * 128:(di + 1) * 128, :],
                                  in_=o_sb[:])

    nc.compile()
    return nc


def _prep_shared(inputs):
    """Host-side layout prep of weight tensors (shared across cores)."""
    import ml_dtypes
    bf16 = np.dtype(ml_dtypes.bfloat16)
    f32 = np.float32

    def c(a, dt):
        return np.ascontiguousarray(a.astype(dt, copy=False))

    w_up, w_gate, w_down = inputs["w_up"], inputs["w_gate"], inputs["w_down"]
    # wug[i][p, k*128+q] = w_up[i*128+q, k*128+p]; cols KT*128.. = w_gate
    wu = w_up.reshape(HT, 128, KT, 128).transpose(0, 3, 2, 1) \
        .reshape(HT, 128, KT * 128)
    wg = w_gate.reshape(HT, 128, KT, 128).transpose(0, 3, 2, 1) \
        .reshape(HT, 128, KT * 128)
    wug = c(np.concatenate([wu, wg], axis=2), bf16)
    wd = c(w_down.reshape(DT, 128, HT, 128).transpose(0, 3, 2, 1)
           .reshape(DT, 128, HT * 128), bf16)

    A_stack = np.concatenate([
        inputs["up_A"].reshape(ER, D),
        inputs["gate_A"].reshape(ER, D)], axis=0)          # [2*ER, D]
    # Ah[p, k*2ER + m] = A_stack[m, k*128+p]
    Ah = c(A_stack.reshape(2 * ER, KT, 128).transpose(2, 1, 0)
           .reshape(128, KT * 2 * ER), bf16)

    up_B_all = (inputs["up_B"].transpose(0, 2, 1).reshape(ER, H)
                * ALPHA).astype(f32)
    gate_B_all = (inputs["gate_B"].transpose(0, 2, 1).reshape(ER, H)
                  * ALPHA).astype(f32)
    uB = up_B_all.reshape(ER, HT, 128).transpose(1, 0, 2)   # [HT, er, h]
    gB = gate_B_all.reshape(ER, HT, 128).transpose(1, 0, 2)
    down_A_all = inputs["down_A"].reshape(ER, H).astype(f32)
    dA = down_A_all.T.reshape(HT, 128, ER)                  # [HT, h, er]
    # ubgd[pair p] = [uB(2p)|gB(2p)|dA(2p)|uB(2p+1)|gB(2p+1)|dA(2p+1)]
    trip = np.concatenate([uB, gB, dA], axis=2)             # [HT, 128, 384]
    ubgd = c(trip.reshape(HT // 2, 2, 128, 384).transpose(0, 2, 1, 3)
             .reshape(HT // 2, 128, 768), bf16)

    down_B_all = (inputs["down_B"].transpose(0, 2, 1).reshape(ER, D)
                  * ALPHA).astype(f32)
    dB = c(down_B_all, bf16)

    gate_wT = inputs["gate_w"].T.astype(f32)               # [D, E]
    gw = c(gate_wT.reshape(KT, 128, E).transpose(1, 0, 2)
           .reshape(128, KT * E), f32)
    gwb = c(gw, bf16)

    eid = (8.0 - (np.arange(128) // R)).astype(f32).reshape(128, 1)
    i8m = np.tile((8.0 - np.arange(E)).astype(f32), (128, 1))
    sel2 = np.zeros((2, 256), f32)
    sel2[0, 0:128] = 1.0
    sel2[1, 128:256] = 1.0

    return dict(wug=wug, wd=wd, Ah=Ah, ubgd=ubgd, dB=dB,
                gw=gw, gwb=gwb, eid=eid, i8m=i8m, sel2=sel2)


def kernel(**inputs):
    import ml_dtypes
    from concourse.bass_utils import run_bass_kernel_spmd

    bf16 = np.dtype(ml_dtypes.bfloat16)
    inputs = {k: np.asarray(v) for k, v in inputs.items()}
    if "nc" not in _cache:
        _cache["nc"] = _build()
    nc = _cache["nc"]

    shared = _prep_shared(inputs)
    x = inputs["x"].astype(np.float32)
    xt = x.reshape(T, D)

    in_maps = []
    for cix in range(NCORES):
        xc = xt[cix * TC:(cix + 1) * TC]                   # [TC, D]
        # packed matmul layout [128, KT*TC]: xT[p, k*TC+t] = xc[t, k*128+p]
        xTp = np.ascontiguousarray(
            xc.reshape(TC, KT, 128).transpose(2, 1, 0).reshape(128, KT * TC))
        b = (cix * TC) // S                                # batch of this core
        xb = xt[b * S:(b + 1) * S]                         # [S, D] whole batch
        off = cix * TC - b * S                             # own slice offset
        rest = np.concatenate([xb[:off], xb[off + TC:]], axis=0)  # [SR, D]
        xrp = np.ascontiguousarray(
            rest.reshape(SR, KT, 128).transpose(2, 1, 0)
            .reshape(128, KT * SR))
        m = dict(shared)
        m["xT"] = xTp.astype(bf16)
        m["xTr"] = xTp
        m["xr"] = xrp.astype(bf16)
        in_maps.append(m)

    res = run_bass_kernel_spmd(nc, in_maps, list(range(NCORES)))
    out = np.empty((T, D), np.float32)
    for cix in range(NCORES):
        out[cix * TC:(cix + 1) * TC, :] = res.results[cix]["outT"].T
    return out.reshape(B, S, D)


# revision 43
# speedup vs baseline: 1.0115x; 1.0115x over previous
"""Trainium2 Bass kernel for nn_MistralMoLoraLayer (MoE-routed LoRA FFN).

Strategy: data-parallel over tokens (8 cores x 256 tokens), base FFN weights
replicated, all-expert LoRA replicated. The per-(batch,slot) softmax over the
sequence axis needs denominators summed over the whole batch; each core
receives the OTHER 768 tokens of its batch (xr) and computes the exp-sums
locally -- no collective, so cores never sync (start-skew stays out of every
core's span).

All heavy GEMMs run in bf16 (own-token router logits stay f32 so top-2
selection is tie-exact vs the f32 reference; rest-of-batch logits feed only
the exp-sum denominator and run bf16). DMA instruction count is minimized
via host-side packed layouts (sequencer + HWDGE cost ~0.6us per DMA): one
DMA for x, one per h-tile for wu|wg, one per h-tile-PAIR for uB|gB|dA, one
per d-tile row of w_down. The router stream (gw/xr/xTr) issues on the
Activation queue so it does not delay the weight stream on the SP queue.
The h-loop is software-pipelined: iteration i emits the base up/gate GEMM
for h-tile i and the LoRA+elementwise chain for h-tile i-2, so the in-order
PE queue has base work queued ahead of every chain matmul that waits on the
DVE stream.

Per-core math (all tiles [h/er/d partitions, tokens free]):
  router: logits = x @ gate_w.T; top-2 (value,index) per token; exp;
          denom[slot] = sum over own 256 + rest 768 tokens of exp;
          weights w_j = exp_j / denom[slot]
  A-proj: UA/GA [E*R=128, t] = stacked up_A/gate_A @ x.T   (one K=128 chain)
  slot-mask trick: Ut_j = UA * M_j where M_j[e*R+r, t] = (sel_j(t)==e);
          lo_up_j[h,t] = (stacked up_B) @ Ut_j  == up_B[sel_j(t)] @ u_{sel_j(t)}
  h_j = silu(U + lo_up_j) * (G + lo_gate_j); ch_j = c_j * h_j
  mixed = ch_0 + ch_1
  v_j[er,t] = (stacked down_A) @ ch_j  (accumulated over h), masked by M_j
  outT[d,t] = w_down-chain @ mixed + (stacked down_B) @ v_0 + ... @ v_1
"""

import numpy as np

# problem constants (hardcoded; kernel.py must be self-contained)
B, S, D, H, E, R, TOPK = 2, 1024, 2048, 5632, 8, 16, 2
ALPHA = 2.0
T = B * S
NCORES = 8
TC = T // NCORES           # 256 tokens per core
KT = D // 128              # 16 k-tiles over D
HT = H // 128              # 44 h-tiles
DT = D // 128              # 16 d-tiles
ER = E * R                 # 128
SR = S - TC                # 768 rest-of-batch tokens for local denom sums
import os as _os
NHOIST = int(_os.environ.get("KNHOIST", "2"))  # h-loop software-pipeline lag
NDEV = int(_os.environ.get("KNDEV", "1"))      # devices declared in the NEFF
WUGB = int(_os.environ.get("KWUGB", "4"))
WDB = int(_os.environ.get("KWDB", "4"))
CHPB = int(_os.environ.get("KCHPB", "3"))

_cache = {}


def _build():
    import concourse.bacc as bacc
    import concourse.bass as bass
    import concourse.mybir as mybir
    import concourse.tile as tile
    from concourse.masks import make_identity

    f32 = mybir.dt.float32
    bf16 = mybir.dt.bfloat16
    AL = mybir.AluOpType
    AF = mybir.ActivationFunctionType

    def fr(ap):
        # f32-stored operand viewed as f32r for fast full-precision-ish matmul
        return ap.bitcast(mybir.dt.float32r)

    # no collectives and no partition-id use remain: build a single-device
    # program (8 independent copies run via shard_map; avoids any comm-group
    # setup at NEFF load)
    nc = bacc.Bacc("TRN2", target_bir_lowering=False, debug=False,
                   num_devices=NDEV)

    # ---- DRAM I/O (host-prepped packed layouts) ----
    d_xT = nc.dram_tensor("xT", [128, KT * TC], bf16, kind="ExternalInput").ap()
    d_xTr = nc.dram_tensor("xTr", [128, KT * TC], f32,
                           kind="ExternalInput").ap()
    d_xr = nc.dram_tensor("xr", [128, KT * SR], bf16,
                          kind="ExternalInput").ap()
    d_gw = nc.dram_tensor("gw", [128, KT * E], f32, kind="ExternalInput").ap()
    d_gwb = nc.dram_tensor("gwb", [128, KT * E], bf16,
                           kind="ExternalInput").ap()
    d_wug = nc.dram_tensor("wug", [HT, 128, 2 * KT * 128], bf16,
                           kind="ExternalInput").ap()
    d_ubgd = nc.dram_tensor("ubgd", [HT // 2, 128, 768], bf16,
                            kind="ExternalInput").ap()
    d_wd = nc.dram_tensor("wd", [DT, 128, HT * 128], bf16,
                          kind="ExternalInput").ap()
    d_A = nc.dram_tensor("Ah", [128, KT * 2 * ER], bf16,
                         kind="ExternalInput").ap()
    d_dB = nc.dram_tensor("dB", [128, D], bf16, kind="ExternalInput").ap()
    d_eid = nc.dram_tensor("eid", [128, 1], f32, kind="ExternalInput").ap()
    d_i8m = nc.dram_tensor("i8m", [128, E], f32, kind="ExternalInput").ap()
    d_sel2 = nc.dram_tensor("sel2", [2, 256], f32, kind="ExternalInput").ap()
    d_out = nc.dram_tensor("outT", [D, TC], f32, kind="ExternalOutput").ap()

    with tile.TileContext(nc) as tc:
        import contextlib
        ctx = contextlib.ExitStack()
        with ctx:
            cpool = ctx.enter_context(tc.tile_pool(name="const", bufs=1))
            wpool = ctx.enter_context(tc.tile_pool(name="wstream", bufs=2))
            spool = ctx.enter_context(tc.tile_pool(name="work", bufs=2))
            pspool = ctx.enter_context(
                tc.tile_pool(name="ps", bufs=1, space="PSUM"))

            # ---- DMA: weight stream on SP queue, router stream on Act ----
            xT_sb = cpool.tile([128, KT * TC], bf16, name="xT_sb")
            nc.sync.dma_start(out=xT_sb[:], in_=d_xT[:])

            XRC = 2                       # xr chunks of 8 k-tiles each
            gw_sb = cpool.tile([128, KT * E], f32, name="gw_sb")
            nc.scalar.dma_start(out=gw_sb[:], in_=d_gw[:])
            gwr_sb = cpool.tile([128, KT * E], bf16, name="gwr_sb")
            nc.scalar.dma_start(out=gwr_sb[:], in_=d_gwb[:])
            xr_t = [wpool.tile([128, 8 * SR], bf16, tag="xr", bufs=2,
                               name=f"xr_t{c}") for c in range(XRC)]
            for c in range(XRC):
                nc.scalar.dma_start(
                    out=xr_t[c][:],
                    in_=d_xr[:, c * 8 * SR:(c + 1) * 8 * SR])
            xTr_sb = cpool.tile([128, KT * TC], f32, name="xTr_sb")
            nc.scalar.dma_start(out=xTr_sb[:], in_=d_xTr[:])
            eid_sb = cpool.tile([128, 1], f32, name="eid_sb")
            nc.scalar.dma_start(out=eid_sb[:], in_=d_eid[:])
            i8m_sb = cpool.tile([128, E], f32, name="i8m_sb")
            nc.scalar.dma_start(out=i8m_sb[:], in_=d_i8m[:])
            sel2_sb = cpool.tile([2, 256], f32, name="sel2_sb")
            nc.scalar.dma_start(out=sel2_sb[:], in_=d_sel2[:])
            dB_sb = cpool.tile([128, D], bf16, name="dB_sb")
            nc.scalar.dma_start(out=dB_sb[:], in_=d_dB[:])

            ident = cpool.tile([128, 128], f32, name="ident")
            make_identity(nc, ident)
            ones_col = cpool.tile([128, 1], f32, name="ones_col")
            nc.vector.memset(ones_col, 1.0)

            mixed = cpool.tile([128, HT * TC], bf16, name="mixed")
            ev_rows = cpool.tile([2, TC], f32, name="ev_rows")
            s_rows = cpool.tile([2, TC], f32, name="s_rows")
            crows = cpool.tile([2, TC], f32, name="crows")
            cb = cpool.tile([128, 2 * TC], bf16, name="cb")
            Mj = cpool.tile([128, 2 * TC], bf16, name="Mj")
            UA = cpool.tile([128, TC], bf16, name="UA")
            GA = cpool.tile([128, TC], bf16, name="GA")
            Ut = cpool.tile([128, 2 * TC], bf16, name="Ut")
            Gt = cpool.tile([128, 2 * TC], bf16, name="Gt")
            vt = cpool.tile([128, 2 * TC], bf16, name="vt")
            Lr = cpool.tile([8, SR], f32, name="Lr")
            ev_acc = cpool.tile([128, 2], f32, name="ev_acc")

            def load_wug(i):
                t = wpool.tile([128, 2 * KT * 128], bf16, tag="wug", bufs=WUGB,
                               name="wug_t")
                nc.sync.dma_start(out=t[:], in_=d_wug[i])
                return t

            def load_ubgd(p):
                t = wpool.tile([128, 768], bf16, tag="ubgd", bufs=3,
                               name="ubgd_t")
                nc.sync.dma_start(out=t[:], in_=d_ubgd[p])
                return t

            def base_gemm(i, wug_t):
                # psUG[:, 0:TC] = up, [TC:2TC] = gate for h-tile i
                psUG = pspool.tile([128, 2 * TC], f32, tag="psUG", bufs=2,
                                   name="psUG")
                for k in range(KT):
                    nc.tensor.matmul(psUG[:, 0:TC],
                                     wug_t[:, k * 128:(k + 1) * 128],
                                     xT_sb[:, k * TC:(k + 1) * TC],
                                     start=(k == 0), stop=(k == KT - 1))
                for k in range(KT):
                    nc.tensor.matmul(
                        psUG[:, TC:2 * TC],
                        wug_t[:, (KT + k) * 128:(KT + k + 1) * 128],
                        xT_sb[:, k * TC:(k + 1) * TC],
                        start=(k == 0), stop=(k == KT - 1))
                U_sb = spool.tile([128, TC], bf16, tag="U_sb",
                                  bufs=NHOIST + 4, name="U_sb")
                nc.scalar.copy(U_sb[:], psUG[:, 0:TC])
                G_sb = spool.tile([128, TC], bf16, tag="G_sb",
                                  bufs=NHOIST + 4, name="G_sb")
                nc.scalar.copy(G_sb[:], psUG[:, TC:2 * TC])
                return U_sb, G_sb

            # ---- hoisted base GEMMs: keep PE busy from the first us while
            #      the router inputs stream in on the Act queue ----
            ug_done = {}                # h-tile -> (U_sb, G_sb)
            ubgd_pre = {}
            wug_pre = [load_wug(i) for i in range(min(2, HT))]
            A_sb = cpool.tile([128, KT * 2 * ER], bf16, name="A_sb")
            nc.sync.dma_start(out=A_sb[:], in_=d_A[:])
            for p in range(2):
                ubgd_pre[p] = load_ubgd(p)
            for i in range(min(2, HT)):
                ug_done[i] = base_gemm(i, wug_pre[i])

            # ---- stacked A-projections (independent of the router; keeps
            #      PE fed while xr/xTr stream in) ----
            psUA = pspool.tile([128, TC], f32, tag="psUG", bufs=2, name="psUA")
            for k in range(KT):
                nc.tensor.matmul(psUA[:],
                                 A_sb[:, k * 2 * ER: k * 2 * ER + ER],
                                 xT_sb[:, k * TC:(k + 1) * TC],
                                 start=(k == 0), stop=(k == KT - 1))
            nc.vector.tensor_copy(UA[:], psUA[:])
            psGA = pspool.tile([128, TC], f32, tag="psUG", bufs=2, name="psGA")
            for k in range(KT):
                nc.tensor.matmul(psGA[:],
                                 A_sb[:, k * 2 * ER + ER:(k + 1) * 2 * ER],
                                 xT_sb[:, k * TC:(k + 1) * TC],
                                 start=(k == 0), stop=(k == KT - 1))
            nc.vector.tensor_copy(GA[:], psGA[:])

            # ---- phase 1a: rest-of-batch logits (bf16, denom-only), E x SR --
            RH = SR // 2
            psr_a = pspool.tile([8, RH], f32, tag="psUG", bufs=2, name="psr_a")
            psr_b = pspool.tile([8, RH], f32, tag="psUG", bufs=2, name="psr_b")
            for k in range(KT):
                xc = xr_t[k // 8]
                sl = (k % 8) * SR
                nc.tensor.matmul(psr_a[:], gwr_sb[:, k * E:(k + 1) * E],
                                 xc[:, sl:sl + RH],
                                 start=(k == 0), stop=(k == KT - 1))
                nc.tensor.matmul(psr_b[:], gwr_sb[:, k * E:(k + 1) * E],
                                 xc[:, sl + RH:sl + SR],
                                 start=(k == 0), stop=(k == KT - 1))
            nc.vector.tensor_copy(Lr[:, 0:RH], psr_a[:])
            nc.vector.tensor_copy(Lr[:, RH:SR], psr_b[:])

            # ---- phase 1b: own-token dance (full f32 logits) ----
            for tt in range(2):
                psL = pspool.tile([128, TC], f32, tag="ps_small", name="psL")
                for k in range(KT):
                    nc.tensor.matmul(
                        psL[:, 0:E],
                        xTr_sb[:, k * TC + tt * 128: k * TC + tt * 128 + 128],
                        gw_sb[:, k * E:(k + 1) * E],
                        start=(k == 0), stop=(k == KT - 1))
                L = spool.tile([128, E], f32, tag="L")
                nc.vector.tensor_copy(L[:], psL[:, 0:E])
                mx1 = spool.tile([128, 1], f32, tag="mx1")
                nc.vector.tensor_reduce(mx1[:], L[:], mybir.AxisListType.X,
                                        AL.max)
                msk = spool.tile([128, E], f32, tag="msk")
                nc.vector.tensor_scalar(msk[:], L[:], mx1[:], None,
                                        AL.is_equal)
                mi = spool.tile([128, E], f32, tag="mi")
                nc.vector.tensor_tensor(mi[:], msk[:], i8m_sb[:], AL.mult)
                svals = spool.tile([128, 2], f32, tag="svals")
                nc.vector.tensor_reduce(svals[:, 0:1], mi[:],
                                        mybir.AxisListType.X, AL.max)
                evals = spool.tile([128, 2], f32, tag="evals")
                nc.scalar.activation(evals[:, 0:1], mx1[:], AF.Exp)
                # mask out slot-0 winner, find second max
                big = spool.tile([128, E], f32, tag="big")
                nc.vector.tensor_scalar(big[:], msk[:], 1e30, None, AL.mult)
                L2 = spool.tile([128, E], f32, tag="L2")
                nc.vector.tensor_tensor(L2[:], L[:], big[:], AL.subtract)
                mx2 = spool.tile([128, 1], f32, tag="mx2")
                nc.vector.tensor_reduce(mx2[:], L2[:], mybir.AxisListType.X,
                                        AL.max)
                msk2 = spool.tile([128, E], f32, tag="msk2")
                nc.vector.tensor_scalar(msk2[:], L2[:], mx2[:], None,
                                        AL.is_equal)
                mi2 = spool.tile([128, E], f32, tag="mi2")
                nc.vector.tensor_tensor(mi2[:], msk2[:], i8m_sb[:], AL.mult)
                nc.vector.tensor_reduce(svals[:, 1:2], mi2[:],
                                        mybir.AxisListType.X, AL.max)
                nc.scalar.activation(evals[:, 1:2], mx2[:], AF.Exp)
                # accumulate exp sums for the denominator
                if tt == 0:
                    nc.vector.tensor_copy(ev_acc[:], evals[:])
                else:
                    nc.vector.tensor_tensor(ev_acc[:], ev_acc[:], evals[:],
                                            AL.add)
                # transpose evals/svals -> rows
                psT = pspool.tile([2, 128], f32, tag="ps_small", name="psT")
                nc.tensor.transpose(psT[:], evals[:], ident[:])
                nc.vector.tensor_copy(ev_rows[:, tt * 128:(tt + 1) * 128],
                                      psT[:])
                psT2 = pspool.tile([2, 128], f32, tag="ps_small", name="psT2")
                nc.tensor.transpose(psT2[:], svals[:], ident[:])
                nc.vector.tensor_copy(s_rows[:, tt * 128:(tt + 1) * 128],
                                      psT2[:])

            # ---- expert masks + masked A-projections: these depend only on
            #      the own-token dance (selection), NOT on the denominators,
            #      so the h-loop's psLO matmuls unblock before the rest dance
            for j in range(2):
                psM = pspool.tile([128, TC], f32, tag="ps_small", name="psM")
                nc.tensor.matmul(psM[:], sel2_sb[:, j * 128:(j + 1) * 128],
                                 s_rows[:], start=True, stop=True)
                nc.vector.tensor_scalar(Mj[:, j * TC:(j + 1) * TC], psM[:],
                                        eid_sb[:], None, AL.is_equal)
                nc.vector.tensor_tensor(Ut[:, j * TC:(j + 1) * TC], UA[:],
                                        Mj[:, j * TC:(j + 1) * TC], AL.mult)
                nc.vector.tensor_tensor(Gt[:, j * TC:(j + 1) * TC], GA[:],
                                        Mj[:, j * TC:(j + 1) * TC], AL.mult)

            # ---- phase 1c: rest-token dance (denominator only); all six
            #      transposes land in ONE psum tile to avoid a PE<->DVE
            #      ping-pong on the single ps_small buffer ----
            NRT = SR // 128
            psLt = pspool.tile([128, 8 * NRT], f32, tag="ps_small",
                               name="psLt")
            for rt in range(NRT):
                nc.tensor.transpose(psLt[:, 8 * rt:8 * rt + 8],
                                    Lr[:, rt * 128:(rt + 1) * 128],
                                    ident[0:8, 0:8])
            L6 = spool.tile([128, 8 * NRT], f32, tag="L6", bufs=1, name="L6")
            nc.vector.tensor_copy(L6[:], psLt[:])
            for rt in range(NRT):
                L = L6[:, 8 * rt:8 * rt + 8]
                mx1 = spool.tile([128, 1], f32, tag="mx1")
                nc.vector.tensor_reduce(mx1[:], L, mybir.AxisListType.X,
                                        AL.max)
                msk = spool.tile([128, E], f32, tag="msk")
                nc.vector.tensor_scalar(msk[:], L, mx1[:], None,
                                        AL.is_equal)
                evals = spool.tile([128, 2], f32, tag="evals")
                nc.scalar.activation(evals[:, 0:1], mx1[:], AF.Exp)
                big = spool.tile([128, E], f32, tag="big")
                nc.vector.tensor_scalar(big[:], msk[:], 1e30, None, AL.mult)
                L2 = spool.tile([128, E], f32, tag="L2")
                nc.vector.tensor_tensor(L2[:], L, big[:], AL.subtract)
                mx2 = spool.tile([128, 1], f32, tag="mx2")
                nc.vector.tensor_reduce(mx2[:], L2[:], mybir.AxisListType.X,
                                        AL.max)
                nc.scalar.activation(evals[:, 1:2], mx2[:], AF.Exp)
                nc.vector.tensor_tensor(ev_acc[:], ev_acc[:], evals[:], AL.add)

            # ---- phase 1d: denominators, reciprocal, routing weights ----
            psd = pspool.tile([2, 1], f32, tag="ps_small", name="psd")
            nc.tensor.matmul(psd[:], ev_acc[:], ones_col[:],
                             start=True, stop=True)
            rcp = cpool.tile([2, 1], f32, name="rcp")
            nc.vector.reciprocal(rcp[:], psd[:])
            nc.vector.tensor_scalar(crows[:], ev_rows[:], rcp[:], None,
                                    AL.mult)
            # broadcast weight rows along partitions via K=2 matmul with a
            # row-selector constant (sel2[:, j*128:(j+1)*128] has row j = 1)
            for j in range(2):
                psB = pspool.tile([128, TC], f32, tag="ps_small", name="psB")
                nc.tensor.matmul(psB[:], sel2_sb[:, j * 128:(j + 1) * 128],
                                 crows[:], start=True, stop=True)
                nc.vector.tensor_copy(cb[:, j * TC:(j + 1) * TC], psB[:])

            # ---- phases 2+5+6: h-tile loop ----
            psV = pspool.tile([128, 2 * TC], f32, tag="psV", name="psV")
            wd_pre = {}                 # di -> prefetched full-row tile

            def load_wd(di):
                t = wpool.tile([128, HT * 128], bf16, tag="wd", bufs=WDB,
                               name="wd_t")
                nc.sync.dma_start(out=t[:], in_=d_wd[di])
                return t

            # software pipeline: iteration i runs the base GEMM for h-tile i
            # and the LoRA/elementwise chain for h-tile j, scheduled with a
            # TAPERED lag: large at the start (chains wait on the router, so
            # queue many base GEMMs ahead of them), shrinking to 1 at the
            # end (minimize the un-overlapped chain tail before the down
            # GEMM can start).
            LAG0 = NHOIST
            sched = {}
            for j_ in range(HT):
                lag = max(1, int(round(LAG0 - (LAG0 - 1) * j_ / 40.0)))
                sched.setdefault(min(j_ + lag, HT), []).append(j_)
            pend = {"v": None}

            def chain(j):
                U_sb, G_sb = ug_done.pop(j)
                ub_t = ubgd_pre[j // 2]
                base = (j % 2) * 384
                uB_t = ub_t[:, base:base + 128]
                gB_t = ub_t[:, base + 128:base + 256]
                dA_t = ub_t[:, base + 256:base + 384]

                if pend["v"] is not None:
                    pv_dA, pv_ch = pend["v"]
                    nc.tensor.matmul(psV[:], pv_dA, pv_ch[:],
                                     start=(j == 1), stop=False,
                                     skip_group_check=True)

                psLO = pspool.tile([128, 4 * TC], f32, tag="psLO", bufs=2,
                                   name="psLO")
                # both slots' c*h in ONE tile so the down_A contraction is a
                # single [128,512] matmul per h-tile
                ch_pair = spool.tile([128, 2 * TC], bf16, tag="chp", bufs=CHPB)
                for sj in range(2):
                    nc.tensor.matmul(psLO[:, (2 * sj) * TC:(2 * sj + 1) * TC],
                                     uB_t, Ut[:, sj * TC:(sj + 1) * TC],
                                     start=True, stop=True)
                    nc.tensor.matmul(
                        psLO[:, (2 * sj + 1) * TC:(2 * sj + 2) * TC],
                        gB_t, Gt[:, sj * TC:(sj + 1) * TC],
                        start=True, stop=True)
                    tu = spool.tile([128, TC], bf16, tag="tu")
                    nc.vector.tensor_tensor(
                        tu[:], U_sb[:],
                        psLO[:, (2 * sj) * TC:(2 * sj + 1) * TC], AL.add)
                    su = spool.tile([128, TC], bf16, tag="su")
                    nc.scalar.activation(su[:], tu[:], AF.Silu)
                    tg = spool.tile([128, TC], bf16, tag="tg")
                    nc.vector.tensor_tensor(
                        tg[:], G_sb[:],
                        psLO[:, (2 * sj + 1) * TC:(2 * sj + 2) * TC], AL.add)
                    hh = spool.tile([128, TC], bf16, tag="hh")
                    nc.vector.tensor_tensor(hh[:], su[:], tg[:], AL.mult)
                    nc.vector.tensor_tensor(ch_pair[:, sj * TC:(sj + 1) * TC],
                                            hh[:],
                                            cb[:, sj * TC:(sj + 1) * TC],
                                            AL.mult)
                nc.vector.tensor_tensor(mixed[:, j * TC:(j + 1) * TC],
                                        ch_pair[:, 0:TC],
                                        ch_pair[:, TC:2 * TC], AL.add)
                pend["v"] = (dA_t, ch_pair)
                # prefetch uB/gB/dA two pairs ahead (after pend_v's reader of
                # the recycled buffer has been emitted)
                if j % 2 == 0 and j // 2 + 2 < HT // 2:
                    ubgd_pre[j // 2 + 2] = load_ubgd(j // 2 + 2)

            for i in range(HT + 1):
                if i < HT:
                    if 38 <= i < 41:
                        wd_pre[i - 38] = load_wd(i - 38)  # prefetch 3 wd rows
                    if i >= 2:
                        ug_done[i] = base_gemm(i, load_wug(i))
                for j in sched.get(i, ()):
                    chain(j)

            pv_dA, pv_ch = pend["v"]
            nc.tensor.matmul(psV[:], pv_dA, pv_ch[:],
                             start=False, stop=True, skip_group_check=True)
            # masked v
            for j in range(2):
                nc.vector.tensor_tensor(vt[:, j * TC:(j + 1) * TC],
                                        psV[:, j * TC:(j + 1) * TC],
                                        Mj[:, j * TC:(j + 1) * TC], AL.mult)

            # ---- phase 7: down GEMM + LoRA-down ----
            for di in range(DT):
                wd_t = wd_pre.pop(di) if di in wd_pre else load_wd(di)
                psO = pspool.tile([128, TC], f32, tag="psUG", bufs=2,
                                  name="psO")
                for hk in range(HT):
                    nc.tensor.matmul(psO[:],
                                     wd_t[:, hk * 128:(hk + 1) * 128],
                                     mixed[:, hk * TC:(hk + 1) * TC],
                                     start=(hk == 0), stop=False,
                                     skip_group_check=True)
                nc.tensor.matmul(psO[:], dB_sb[:, di * 128:(di + 1) * 128],
                                 vt[:, 0:TC], start=False, stop=False,
                                 skip_group_check=True)
                nc.tensor.matmul(psO[:], dB_sb[:, di * 128:(di + 1) * 128],
                                 vt[:, TC:2 * TC], start=False, stop=True,
                                 skip_group_check=True)
                o_sb = spool.tile([128, TC], f32, tag="o_sb")
                nc.scalar.copy(o_sb[:], psO[:])
                nc.sync.dma_start(out=d_out[di * 128:(di + 1) * 128, :],
                                  in_=o_sb[:])

    nc.compile()
    return nc


def _prep_shared(inputs):
    """Host-side layout prep of weight tensors (shared across cores)."""
    import ml_dtypes
    bf16 = np.dtype(ml_dtypes.bfloat16)
    f32 = np.float32

    def c(a, dt):
        return np.ascontiguousarray(a.astype(dt, copy=False))

    w_up, w_gate, w_down = inputs["w_up"], inputs["w_gate"], inputs["w_down"]
    # wug[i][p, k*128+q] = w_up[i*128+q, k*128+p]; cols KT*128.. = w_gate
    wu = w_up.reshape(HT, 128, KT, 128).transpose(0, 3, 2, 1) \
        .reshape(HT, 128, KT * 128)
    wg = w_gate.reshape(HT, 128, KT, 128).transpose(0, 3, 2, 1) \
        .reshape(HT, 128, KT * 128)
    wug = c(np.concatenate([wu, wg], axis=2), bf16)
    wd = c(w_down.reshape(DT, 128, HT, 128).transpose(0, 3, 2, 1)
           .reshape(DT, 128, HT * 128), bf16)

    A_stack = np.concatenate([
        inputs["up_A"].reshape(ER, D),
        inputs["gate_A"].reshape(ER, D)], axis=0)          # [2*ER, D]
    # Ah[p, k*2ER + m] = A_stack[m, k*128+p]
    Ah = c(A_stack.reshape(2 * ER, KT, 128).transpose(2, 1, 0)
           .reshape(128, KT * 2 * ER), bf16)

    up_B_all = (inputs["up_B"].transpose(0, 2, 1).reshape(ER, H)
                * ALPHA).astype(f32)
    gate_B_all = (inputs["gate_B"].transpose(0, 2, 1).reshape(ER, H)
                  * ALPHA).astype(f32)
    uB = up_B_all.reshape(ER, HT, 128).transpose(1, 0, 2)   # [HT, er, h]
    gB = gate_B_all.reshape(ER, HT, 128).transpose(1, 0, 2)
    down_A_all = inputs["down_A"].reshape(ER, H).astype(f32)
    dA = down_A_all.T.reshape(HT, 128, ER)                  # [HT, h, er]
    # ubgd[pair p] = [uB(2p)|gB(2p)|dA(2p)|uB(2p+1)|gB(2p+1)|dA(2p+1)]
    trip = np.concatenate([uB, gB, dA], axis=2)             # [HT, 128, 384]
    ubgd = c(trip.reshape(HT // 2, 2, 128, 384).transpose(0, 2, 1, 3)
             .reshape(HT // 2, 128, 768), bf16)

    down_B_all = (inputs["down_B"].transpose(0, 2, 1).reshape(ER, D)
                  * ALPHA).astype(f32)
    dB = c(down_B_all, bf16)

    gate_wT = inputs["gate_w"].T.astype(f32)               # [D, E]
    gw = c(gate_wT.reshape(KT, 128, E).transpose(1, 0, 2)
           .reshape(128, KT * E), f32)
    gwb = c(gw, bf16)

    eid = (8.0 - (np.arange(128) // R)).astype(f32).reshape(128, 1)
    i8m = np.tile((8.0 - np.arange(E)).astype(f32), (128, 1))
    sel2 = np.zeros((2, 256), f32)
    sel2[0, 0:128] = 1.0
    sel2[1, 128:256] = 1.0

    return dict(wug=wug, wd=wd, Ah=Ah, ubgd=ubgd, dB=dB,
                gw=gw, gwb=gwb, eid=eid, i8m=i8m, sel2=sel2)


def _make_in_maps(inputs):
    import ml_dtypes
    bf16 = np.dtype(ml_dtypes.bfloat16)
    shared = _prep_shared(inputs)
    x = inputs["x"].astype(np.float32)
    xt = x.reshape(T, D)

    in_maps = []
    for cix in range(NCORES):
        xc = xt[cix * TC:(cix + 1) * TC]                   # [TC, D]
        # packed matmul layout [128, KT*TC]: xT[p, k*TC+t] = xc[t, k*128+p]
        xTp = np.ascontiguousarray(
            xc.reshape(TC, KT, 128).transpose(2, 1, 0).reshape(128, KT * TC))
        b = (cix * TC) // S                                # batch of this core
        xb = xt[b * S:(b + 1) * S]                         # [S, D] whole batch
        off = cix * TC - b * S                             # own slice offset
        rest = np.concatenate([xb[:off], xb[off + TC:]], axis=0)  # [SR, D]
        xrp = np.ascontiguousarray(
            rest.reshape(SR, KT, 128).transpose(2, 1, 0)
            .reshape(128, KT * SR))
        m = dict(shared)
        m["xT"] = xTp.astype(bf16)
        m["xTr"] = xTp
        m["xr"] = xrp.astype(bf16)
        in_maps.append(m)
    return in_maps


def kernel(**inputs):
    from concourse.bass_utils import run_bass_kernel_spmd

    inputs = {k: np.asarray(v) for k, v in inputs.items()}
    if "nc" not in _cache:
        _cache["nc"] = _build()
    nc = _cache["nc"]

    in_maps = _make_in_maps(inputs)
    res = run_bass_kernel_spmd(nc, in_maps, list(range(NCORES)))
    out = np.empty((T, D), np.float32)
    for cix in range(NCORES):
        out[cix * TC:(cix + 1) * TC, :] = res.results[cix]["outT"].T
    return out.reshape(B, S, D)
